# revision 4
# baseline (speedup 1.0000x reference)
"""Trainium2 Bass kernel for nn_HAN_Integrated (GatedGraph message passing).

Math per iteration (reference.py):
    act[e]  = edge_matrix[e].T @ h + ba            # [N,S] per edge type
    z       = sigmoid(sum_e act[e] @ wz[e] + h @ uz)
    r       = sigmoid(sum_e act[e] @ wr[e] + h @ ur)
    hh      = tanh  (sum_e act[e] @ wh[e] + (r*h) @ uh)
    h       = (1-z)*h + r*hh                        # 10 iterations

Sharding: columns (n) of the [E,N,N] adjacency are row-partitioned across
8 cores (NL=375 nodes per core). Each core computes act/z/r/hh/h_new for
its 375 nodes, then an AllGather rebuilds the full [N,S] h each iteration.

All on-chip compute is done in a TRANSPOSED [S, n_local] layout so that:
  - stage A:  out[s,n] += sum_m h[m,s] * em[e,m,n]   (lhsT = h tile, rhs = em tile)
  - stage B:  out[k,n] += sum_s w[e,s,k] * actT[s,n] (lhsT = w[e],   rhs = actT)
both keep the streamed edge matrix as the *moving* operand.

The recurrence is chaotic (error amplification ~150x over 10 iterations),
so bf16 compute fails; matmuls run as float32r (full-rate fp32 datapath)
or plain float32 (4x slower, exact) selected by STAGE_F32R.
"""

import sys

for _p in ("/opt/trn_rl_repo", "/opt/pypackages"):
    if _p not in sys.path:
        sys.path.insert(0, _p)

import numpy as np

import concourse.bacc as bacc
import concourse.mybir as mybir
from concourse import masks
from concourse.bass_utils import run_bass_kernel_spmd
from concourse.tile import TileContext

N, S, E = 3000, 64, 12
NCORES = 8
NL = N // NCORES          # 375 nodes per core
MT = 125                  # contraction (m) tile -> partition dim
T = N // MT               # 24 m-tiles
ITERS = 10
F32 = mybir.dt.float32
AF = mybir.ActivationFunctionType

STAGE_F32R = False         # float32r matmuls (full rate). False -> exact fp32 (4x slower PE).
EM_BUFS = 3               # SBUF double/triple buffering of the 4.5MB edge slices


def build_module(f32r: bool = STAGE_F32R, em_bufs: int = EM_BUFS):
    nc = bacc.Bacc("TRN2", target_bir_lowering=False, debug=False, num_devices=NCORES)

    if f32r:
        def cast(ap):
            return ap.bitcast(mybir.dt.float32r)
    else:
        def cast(ap):
            return ap

    em_d = nc.dram_tensor("em", [E, MT, T * NL], F32, kind="ExternalInput")
    h0_d = nc.dram_tensor("h0", [N, S], F32, kind="ExternalInput")
    h0T_d = nc.dram_tensor("h0T", [S, NL], F32, kind="ExternalInput")
    w_d = {nm: nc.dram_tensor(nm, [S, E * S], F32, kind="ExternalInput")
           for nm in ("wz", "wr", "wh")}
    u_d = {nm: nc.dram_tensor(nm, [S, S], F32, kind="ExternalInput")
           for nm in ("uz", "ur", "uh")}
    ba_d = nc.dram_tensor("ba", [S, 1], F32, kind="ExternalInput")
    out_d = nc.dram_tensor("out", [NL, S], F32, kind="ExternalOutput")

    rg = [list(range(NCORES))]

    with TileContext(nc) as tc:
        with (
            tc.tile_pool(name="const", bufs=1) as cpool,
            tc.tile_pool(name="empool", bufs=em_bufs) as empool,
            tc.tile_pool(name="hpool", bufs=2) as hpool,
            tc.tile_pool(name="htpool", bufs=2) as htpool,
            tc.tile_pool(name="apool", bufs=3) as apool,
            tc.tile_pool(name="gpool", bufs=2) as gpool,
            tc.tile_pool(name="papool", bufs=2, space="PSUM") as papool,
            tc.tile_pool(name="pgpool", bufs=1, space="PSUM") as pgpool,
            tc.tile_pool(name="trpool", bufs=3, space="PSUM") as trpool,
            tc.tile_pool(name="dpool", bufs=2, space="DRAM") as dpool,
            tc.tile_pool(name="dspool", bufs=2, space="DRAM") as dspool,
        ):
            ident = cpool.tile([128, 128], F32, name="ident")
            masks.make_identity(nc, ident[:])

            w_sb = {}
            for nm in ("wz", "wr", "wh"):
                t = cpool.tile([S, E * S], F32, name=f"{nm}_sb")
                nc.sync.dma_start(out=t[:], in_=w_d[nm][:])
                w_sb[nm] = t
            u_sb = {}
            for nm in ("uz", "ur", "uh"):
                t = cpool.tile([S, S], F32, name=f"{nm}_sb")
                nc.sync.dma_start(out=t[:], in_=u_d[nm][:])
                u_sb[nm] = t
            ba_sb = cpool.tile([S, 1], F32, name="ba_sb")
            nc.sync.dma_start(out=ba_sb[:], in_=ba_d[:])

            # h for the local nodes, transposed [S, NL], fp32 (stays on chip)
            hT_cur = htpool.tile([S, NL], F32, tag="hT")
            nc.scalar.dma_start(out=hT_cur[:], in_=h0T_d[:])

            ag_out = None
            for it in range(ITERS):
                # full h in [m, s] tiles: partition p holds node t*MT+p at cols t*S..
                h_cur = hpool.tile([MT, T * S], F32, tag="h")
                src = h0_d[:] if it == 0 else ag_out[:]
                nc.scalar.dma_start(
                    out=h_cur[:].rearrange("p (t s) -> p t s", t=T),
                    in_=src.rearrange("(t p) s -> p t s", p=MT),
                )

                pz = pgpool.tile([S, NL], F32, tag="pz")
                pr = pgpool.tile([S, NL], F32, tag="pr")
                ph = pgpool.tile([S, NL], F32, tag="ph")

                for e in range(E):
                    em_t = empool.tile([MT, T * NL], F32, tag="em")
                    nc.sync.dma_start(out=em_t[:], in_=em_d[e])
                    pa = papool.tile([S, NL], F32, tag="pa")
                    for t in range(T):
                        nc.tensor.matmul(
                            pa[:],
                            lhsT=cast(h_cur[:, t * S:(t + 1) * S]),
                            rhs=cast(em_t[:, t * NL:(t + 1) * NL]),
                            start=(t == 0),
                            stop=(t == T - 1),
                        )
                    # act[e]^T to SBUF with the ba bias folded in
                    actT = apool.tile([S, NL], F32, tag="act")
                    nc.vector.tensor_scalar_add(actT[:], pa[:], ba_sb[:, 0:1])
                    for nm, pg in (("wz", pz), ("wr", pr), ("wh", ph)):
                        nc.tensor.matmul(
                            pg[:],
                            lhsT=cast(w_sb[nm][:, e * S:(e + 1) * S]),
                            rhs=cast(actT[:]),
                            start=(e == 0),
                            stop=False,
                            skip_group_check=True,
                        )

                nc.tensor.matmul(pz[:], lhsT=cast(u_sb["uz"][:]), rhs=cast(hT_cur[:]),
                                 start=False, stop=True, skip_group_check=True)
                nc.tensor.matmul(pr[:], lhsT=cast(u_sb["ur"][:]), rhs=cast(hT_cur[:]),
                                 start=False, stop=True, skip_group_check=True)

                # omz = 1 - z = sigmoid(-z_pre); r = sigmoid(r_pre)
                omz = gpool.tile([S, NL], F32, tag="omz")
                nc.scalar.activation(omz[:], pz[:], AF.Sigmoid, scale=-1.0)
                r_sb = gpool.tile([S, NL], F32, tag="r")
                nc.scalar.activation(r_sb[:], pr[:], AF.Sigmoid)

                rh = gpool.tile([S, NL], F32, tag="rh")
                nc.vector.tensor_mul(rh[:], r_sb[:], hT_cur[:])
                nc.tensor.matmul(ph[:], lhsT=cast(u_sb["uh"][:]), rhs=cast(rh[:]),
                                 start=False, stop=True, skip_group_check=True)
                hh = gpool.tile([S, NL], F32, tag="hh")
                nc.scalar.activation(hh[:], ph[:], AF.Tanh)

                # h_new = (1-z)*h + r*hh
                m1 = gpool.tile([S, NL], F32, tag="m1")
                nc.vector.tensor_mul(m1[:], omz[:], hT_cur[:])
                m2 = gpool.tile([S, NL], F32, tag="m2")
                nc.vector.tensor_mul(m2[:], r_sb[:], hh[:])
                hT_new = htpool.tile([S, NL], F32, tag="hT")
                nc.vector.tensor_add(hT_new[:], m1[:], m2[:])
                hT_cur = hT_new

                # transpose [S,NL] -> [NL,S] through PE, bounce via SBUF
                hn_sb = gpool.tile([MT, 3 * S], F32, tag="hn")
                if it < ITERS - 1:
                    ag_in = dpool.tile([NL, S], F32, tag="ag_in")
                    for c in range(3):
                        ptr = trpool.tile([MT, S], F32, tag="ptr")
                        nc.tensor.transpose(ptr[:], hT_new[:, c * MT:(c + 1) * MT],
                                            ident[:S, :S])
                        nc.scalar.copy(hn_sb[:, c * S:(c + 1) * S], ptr[:])
                        nc.scalar.dma_start(out=ag_in[c * MT:(c + 1) * MT, :],
                                            in_=hn_sb[:, c * S:(c + 1) * S])
                    ag_out = dspool.tile([N, S], F32, tag="ag_out",
                                         addr_space="Shared")
                    nc.gpsimd.collective_compute(
                        "AllGather",
                        mybir.AluOpType.bypass,
                        replica_groups=rg,
                        ins=[ag_in[:].opt()],
                        outs=[ag_out[:].opt()],
                    )
                else:
                    for c in range(3):
                        ptr = trpool.tile([MT, S], F32, tag="ptr")
                        nc.tensor.transpose(ptr[:], hT_new[:, c * MT:(c + 1) * MT],
                                            ident[:S, :S])
                        nc.scalar.copy(hn_sb[:, c * S:(c + 1) * S], ptr[:])
                        nc.scalar.dma_start(out=out_d[c * MT:(c + 1) * MT, :],
                                            in_=hn_sb[:, c * S:(c + 1) * S])

    nc.finalize()
    return nc


def make_in_maps(x, edge_matrix, ba, wz, wr, wh, uz, ur, uh):
    x = np.ascontiguousarray(np.asarray(x, np.float32))
    em = np.asarray(edge_matrix, np.float32)
    w_h = {nm: np.ascontiguousarray(np.asarray(w, np.float32).transpose(1, 0, 2)
                                    .reshape(S, E * S))
           for nm, w in (("wz", wz), ("wr", wr), ("wh", wh))}
    u_h = {nm: np.ascontiguousarray(np.asarray(u, np.float32))
           for nm, u in (("uz", uz), ("ur", ur), ("uh", uh))}
    ba_h = np.ascontiguousarray(np.asarray(ba, np.float32).reshape(S, 1))

    in_maps = []
    for rr in range(NCORES):
        n0 = rr * NL
        shard = em[:, :, n0:n0 + NL].reshape(E, T, MT, NL)
        shard = np.ascontiguousarray(shard.transpose(0, 2, 1, 3)).reshape(E, MT, T * NL)
        in_maps.append({
            "em": shard,
            "h0": x,
            "h0T": np.ascontiguousarray(x[n0:n0 + NL].T),
            "wz": w_h["wz"], "wr": w_h["wr"], "wh": w_h["wh"],
            "uz": u_h["uz"], "ur": u_h["ur"], "uh": u_h["uh"],
            "ba": ba_h,
        })
    return in_maps


_NC_CACHE = {}


def get_nc(f32r: bool = STAGE_F32R, em_bufs: int = EM_BUFS):
    key = (f32r, em_bufs)
    if key not in _NC_CACHE:
        _NC_CACHE[key] = build_module(f32r, em_bufs)
    return _NC_CACHE[key]


def run(inputs, trace=False, f32r: bool = STAGE_F32R, em_bufs: int = EM_BUFS):
    in_maps = make_in_maps(
        inputs["x"], inputs["edge_matrix"], inputs["ba"],
        inputs["wz"], inputs["wr"], inputs["wh"],
        inputs["uz"], inputs["ur"], inputs["uh"],
    )
    res = run_bass_kernel_spmd(get_nc(f32r, em_bufs), in_maps,
                               core_ids=list(range(NCORES)), trace=trace)
    out = np.concatenate([res.results[r]["out"] for r in range(NCORES)], axis=0)
    return np.ascontiguousarray(out, dtype=np.float32), res


def kernel(x, edge_matrix, ba, wz, wr, wh, uz, ur, uh, iteration):
    assert int(iteration) == ITERS, f"kernel hardcodes {ITERS} iterations"
    out, _ = run({"x": x, "edge_matrix": edge_matrix, "ba": ba,
                  "wz": wz, "wr": wr, "wh": wh,
                  "uz": uz, "ur": ur, "uh": uh})
    return out
